# revision 64
# baseline (speedup 1.0000x reference)
"""NeighborAttention (B=4, N=4096, K=32, C=128, H=4) on 8 Trainium2 cores.

v4 design (neighbor compaction + engine rebalance), ~141us vs the
272.9us v2 baseline:

  Host packs each node's unmasked neighbors first (attention is
  permutation-invariant over neighbors), sorts nodes globally by
  neighbor count, and deals them round-robin to the 8 cores. Chunks of
  CN=512 nodes get an adaptive per-chunk neighbor capacity K' (even,
  typically (14,16,18,26)) so ~42% of all per-neighbor compute and DMA
  disappears versus uniform K=32. Chunks run largest-K' first.

  Per chunk (j-major layout [c, j, n], bf16, K'=kj):
    AB: PE kt pairs = Wk'@et ([C,1024] PSUM tiles); DVE prod = kt*q
        (1024-wide, q duplicated to [C,1024]); PE s pairs = Hrep@prod
        and ACT e = exp(s) (1024-wide) interleaved one pair behind.
        Score pairs borrow the kt PSUM ring (bufs=3, slots alternate
        kt -> s usage) to stay within the 8 PSUM banks.
    CD: PE vt pairs = Wv'@et; ACT copies vt->SBUF bf16 so the DVE
        uv = e*vt multiply runs in 2x mode (first/last pairs read PSUM
        directly to avoid the ACT queue backlog); PE z += I@e_j and
        usum += I@uv_j identity-accumulations interleaved; the pad
        correction (cnt-K') is folded into the z accumulation as one
        extra identity matmul.
    tail (deferred past the next chunk's AB for overlap):
        DVE in-place bf16 max tree over uv slabs -> umax;
        zc = clamp(z); rz = reciprocal_approx_accurate(zc);
        wsn = usum*rz, mxn = umax*rz (per-head 1/z scaling must precede
        W_O, which mixes heads); PE o = Wos@wsn + Wo3@mxn; ACT copy;
        DMA out.

  attn sums to exactly 1 so the mean/sum W_O blocks fold on the host;
  compaction pad slots have et=0 and contribute exp(0)=1 to z, removed
  by the folded correction term. Each chunk's q is projected one chunk
  ahead (chunk 0's in the prologue) so its ACT copies precede the vc
  backlog. The final chunk's max tree writes level 1 into the dead et
  tile so it does not WAR-wait on the PE usum matmuls still reading uv.
"""
import numpy as np
import ml_dtypes
import concourse.bass as bass
import concourse.bacc as bacc
import concourse.mybir as mybir
from concourse import tile
from concourse.bass_utils import run_bass_kernel_spmd

F32 = mybir.dt.float32
BF16 = mybir.dt.bfloat16
EXP = mybir.ActivationFunctionType.Exp

K = 32
C = 128
H = 4
D = 32
NCORES = 8
CN = 512              # nodes per chunk

_NC_CACHE = {}


def build_nc(sched):
    """sched: tuple of per-chunk neighbor capacities (even ints)."""
    key = tuple(sched)
    if key in _NC_CACHE:
        return _NC_CACHE[key]
    nchunks = len(sched)
    nloc = nchunks * CN
    ncols = sum(sched) * CN
    offs = np.cumsum([0] + [k * CN for k in sched]).tolist()

    nc = bacc.Bacc()
    et_d = nc.dram_tensor("et", [C, ncols], BF16, kind="ExternalInput")
    xt_d = nc.dram_tensor("xt", [C, nloc], BF16, kind="ExternalInput")
    wq_d = nc.dram_tensor("wq", [C, C], BF16, kind="ExternalInput")
    wk_d = nc.dram_tensor("wk", [C, C], BF16, kind="ExternalInput")
    wv_d = nc.dram_tensor("wv", [C, C], BF16, kind="ExternalInput")
    hr_d = nc.dram_tensor("hr", [C, C], BF16, kind="ExternalInput")
    id_d = nc.dram_tensor("idn", [C, C], BF16, kind="ExternalInput")
    wos_d = nc.dram_tensor("wos", [C, C], BF16, kind="ExternalInput")
    wo3_d = nc.dram_tensor("wo3", [C, C], BF16, kind="ExternalInput")
    mc_d = nc.dram_tensor("mc", [C, nloc], BF16, kind="ExternalInput")
    out_d = nc.dram_tensor("out", [C, nloc], F32, kind="ExternalOutput")

    with tile.TileContext(nc) as tc:
        with tc.tile_pool(name="wts", bufs=1) as wpool, \
             tc.tile_pool(name="glob", bufs=1) as gpool, \
             tc.tile_pool(name="etp", bufs=2) as etpool, \
             tc.tile_pool(name="s1p", bufs=2) as s1pool, \
             tc.tile_pool(name="s2p", bufs=2) as s2pool, \
             tc.tile_pool(name="sm", bufs=2) as smpool, \
             tc.tile_pool(name="pkv", bufs=3, space="PSUM") as pkv, \
             tc.tile_pool(name="psz", bufs=1, space="PSUM") as psz, \
             tc.tile_pool(name="psu", bufs=1, space="PSUM") as psu:

            w_q = wpool.tile([C, C], BF16, tag="wq")
            w_k = wpool.tile([C, C], BF16, tag="wk")
            w_v = wpool.tile([C, C], BF16, tag="wv")
            w_h = wpool.tile([C, C], BF16, tag="wh")
            w_i = wpool.tile([C, C], BF16, tag="wi")
            w_os = wpool.tile([C, C], BF16, tag="wos")
            w_o3 = wpool.tile([C, C], BF16, tag="wo3")
            xt_sb = gpool.tile([C, nloc], BF16, tag="xt")
            mc_sb = gpool.tile([C, nloc], BF16, tag="mc")

            # chunk processing order: largest K' first, so the biggest
            # tree/epilogue tails overlap later chunks and the final
            # exposed tail is the smallest one
            corder = sorted(range(nchunks), key=lambda c: -sched[c])

            # DMA priority: what phase A of the first chunk needs comes
            # first
            nc.sync.dma_start(w_q[:], wq_d[:])
            nc.sync.dma_start(xt_sb[:], xt_d[:])
            nc.sync.dma_start(w_k[:], wk_d[:])

            def load_et(ch, pieces=2):
                kj = sched[ch]
                et_sb = etpool.tile([C, kj * CN], BF16, tag="et")
                w = kj * CN
                bnds = [w * i // pieces for i in range(pieces + 1)]
                for a, b in zip(bnds, bnds[1:]):
                    nc.sync.dma_start(et_sb[:, a:b],
                                      et_d[:, offs[ch] + a:offs[ch] + b])
                return et_sb

            et_tiles = {corder[0]: load_et(corder[0], pieces=4)}
            for t, dd in ((w_h, hr_d), (w_v, wv_d), (w_i, id_d),
                          (w_os, wos_d), (w_o3, wo3_d)):
                nc.sync.dma_start(t[:], dd[:])
            nc.sync.dma_start(mc_sb[:], mc_d[:])

            # q projection for a chunk, duplicated to [C, 2CN] so the
            # paired 1024-wide prod multiplies can broadcast it over both
            # slabs; issued one chunk ahead so the ACT copies precede the
            # vc backlog (chunk 0's is issued here in the prologue)
            def emit_q(c2):
                q_ps = psz.tile([C, CN], F32, tag="z", name="q_ps")
                nc.tensor.matmul(q_ps[:], w_q[:],
                                 xt_sb[:, c2 * CN:(c2 + 1) * CN],
                                 start=True, stop=True)
                q2 = smpool.tile([C, 2 * CN], F32, tag="q", bufs=2,
                                 name="q2")
                nc.scalar.copy(q2[:, :CN], q_ps[:])
                nc.scalar.copy(q2[:, CN:], q_ps[:])
                return q2

            q_next = emit_q(corder[0])
            pending_tail = None

            for ci in range(nchunks):
                ch = corder[ci]
                kj = sched[ch]
                n0 = ch * CN
                et_sb = et_tiles.pop(ch)
                if ci + 1 < nchunks:
                    et_tiles[corder[ci + 1]] = load_et(corder[ci + 1])

                q_sb = q_next

                prod = s1pool.tile([C, kj * CN], BF16, tag="s1")
                e_ch = s2pool.tile([C, kj * CN], BF16, tag="s2")

                # AB: k-projection (paired into [C,1024] PSUM tiles, one
                # wide DVE multiply per pair) with score matmuls + exp
                # interleaved one pair behind. Score pairs borrow the same
                # kv PSUM ring (each slot alternates kt -> s usage), which
                # keeps exp 1024-wide within the 8-bank budget.
                def emit_score_pair(j):
                    s_ps = pkv.tile([C, 2 * CN], F32, tag="kv")
                    nc.tensor.matmul(s_ps[:, :CN], w_h[:],
                                     prod[:, j * CN:(j + 1) * CN],
                                     start=True, stop=True)
                    nc.tensor.matmul(s_ps[:, CN:], w_h[:],
                                     prod[:, (j + 1) * CN:(j + 2) * CN],
                                     start=True, stop=True)
                    nc.scalar.activation(e_ch[:, j * CN:(j + 2) * CN],
                                         s_ps[:], EXP)

                for jp in range(kj // 2):
                    j = 2 * jp
                    kt2 = pkv.tile([C, 2 * CN], F32, tag="kv")
                    nc.tensor.matmul(kt2[:, :CN], w_k[:],
                                     et_sb[:, j * CN:(j + 1) * CN],
                                     start=True, stop=True)
                    nc.tensor.matmul(kt2[:, CN:], w_k[:],
                                     et_sb[:, (j + 1) * CN:(j + 2) * CN],
                                     start=True, stop=True)
                    nc.vector.tensor_mul(prod[:, j * CN:(j + 2) * CN],
                                         kt2[:], q_sb[:])
                    if jp >= 1:
                        emit_score_pair(j - 2)
                emit_score_pair(kj - 2)

                # previous chunk's max tree + epilogue runs here, overlapped
                # with this chunk's CD phase
                if pending_tail is not None:
                    pending_tail()
                if ci + 1 < nchunks:
                    q_next = emit_q(corder[ci + 1])

                # CD: v-projection (paired), ACT PSUM->SBUF bf16 copy so the
                # DVE multiply runs in 2x mode, z/usum identity accumulation
                # interleaved
                uv = prod
                z_ps = psz.tile([C, CN], F32, tag="z")
                u_ps = psu.tile([C, CN], F32, tag="u")

                npair = kj // 2
                for jp in range(npair):
                    j = 2 * jp
                    vt2 = pkv.tile([C, 2 * CN], F32, tag="kv")
                    nc.tensor.matmul(vt2[:, :CN], w_v[:],
                                     et_sb[:, j * CN:(j + 1) * CN],
                                     start=True, stop=True)
                    nc.tensor.matmul(vt2[:, CN:], w_v[:],
                                     et_sb[:, (j + 1) * CN:(j + 2) * CN],
                                     start=True, stop=True)
                    if jp == 0 or jp == npair - 1:
                        # direct PSUM multiply: no ACT-queue dependency, so
                        # the DVE starts immediately and the kv ring frees
                        # without waiting on the (backlogged) ACT copies
                        nc.vector.tensor_mul(uv[:, j * CN:(j + 2) * CN],
                                             e_ch[:, j * CN:(j + 2) * CN],
                                             vt2[:])
                    else:
                        vc = smpool.tile([C, 2 * CN], BF16, tag="vc", bufs=4)
                        nc.scalar.copy(vc[:], vt2[:])
                        nc.vector.tensor_mul(uv[:, j * CN:(j + 2) * CN],
                                             e_ch[:, j * CN:(j + 2) * CN],
                                             vc[:])
                    nc.tensor.matmul(z_ps[:], w_i[:],
                                     e_ch[:, j * CN:(j + 1) * CN],
                                     start=(j == 0), stop=False,
                                     skip_group_check=True)
                    nc.tensor.matmul(z_ps[:], w_i[:],
                                     e_ch[:, (j + 1) * CN:(j + 2) * CN],
                                     start=False, stop=False,
                                     skip_group_check=True)
                    if jp >= 1:
                        nc.tensor.matmul(u_ps[:], w_i[:],
                                         uv[:, (j - 2) * CN:(j - 1) * CN],
                                         start=(jp == 1), stop=False,
                                         skip_group_check=True)
                        nc.tensor.matmul(u_ps[:], w_i[:],
                                         uv[:, (j - 1) * CN:j * CN],
                                         start=False, stop=False,
                                         skip_group_check=True)
                nc.tensor.matmul(u_ps[:], w_i[:],
                                 uv[:, (kj - 2) * CN:(kj - 1) * CN],
                                 start=(kj == 2), stop=False,
                                 skip_group_check=True)
                nc.tensor.matmul(u_ps[:], w_i[:],
                                 uv[:, (kj - 1) * CN:kj * CN],
                                 start=False, stop=True,
                                 skip_group_check=True)

                # fold the pad correction into the z accumulation:
                # mc holds (cnt - K') so this subtracts the pad count
                nc.tensor.matmul(z_ps[:], w_i[:], mc_sb[:, n0:n0 + CN],
                                 start=False, stop=True,
                                 skip_group_check=True)

                def make_tail(kj, n0, uv, z_ps, u_ps, scratch=None):
                    def tail():
                        # T: max tree over uv slabs (DVE, bf16 2x). For the
                        # final chunk the first level writes into the dead et
                        # tile instead of in-place: writing uv would WAR-wait
                        # on the PE usum matmuls still reading it, exposing
                        # ~4us of serial tail at the very end.
                        tre = uv
                        scr = scratch
                        w = kj
                        while w > 1:
                            hw = w // 2
                            dst = scr if scr is not None else tre
                            nc.vector.tensor_max(dst[:, :hw * CN],
                                                 tre[:, :hw * CN],
                                                 tre[:, hw * CN:2 * hw * CN])
                            if w % 2:
                                nc.vector.tensor_max(dst[:, :CN],
                                                     dst[:, :CN],
                                                     tre[:, (w - 1) * CN:w * CN])
                            tre = dst
                            scr = None  # subsequent levels in-place
                            w = hw

                        # E: scale aggregates by 1/z per head BEFORE W_O
                        zcl = smpool.tile([C, CN], F32, tag="zcl", bufs=1)
                        nc.vector.tensor_scalar_max(zcl[:], z_ps[:], 1e-20)
                        rz = smpool.tile([C, CN], F32, tag="rz")
                        rscr = smpool.tile([C, CN], F32, tag="rscr", bufs=1)
                        nc.vector.reciprocal_approx_accurate(rz[:], zcl[:],
                                                             rscr[:])

                        wsn = smpool.tile([C, CN], BF16, tag="wsn")
                        nc.vector.tensor_mul(wsn[:], u_ps[:], rz[:])
                        mxn = smpool.tile([C, CN], BF16, tag="mxn")
                        nc.vector.tensor_mul(mxn[:], tre[:, :CN], rz[:])

                        o_ps = psu.tile([C, CN], F32, tag="u")
                        nc.tensor.matmul(o_ps[:], w_os[:], wsn[:],
                                         start=True, stop=False)
                        nc.tensor.matmul(o_ps[:], w_o3[:], mxn[:],
                                         start=False, stop=True)
                        o_sb = smpool.tile([C, CN], F32, tag="osb")
                        nc.scalar.copy(o_sb[:], o_ps[:])
                        nc.sync.dma_start(out_d[:, n0:n0 + CN], o_sb[:])
                    return tail

                pending_tail = make_tail(
                    kj, n0, uv, z_ps, u_ps,
                    scratch=et_sb if ci == nchunks - 1 else None)

            pending_tail()

    nc.compile()
    _NC_CACHE[key] = nc
    return nc


def _perm_dh(w):
    """torch-layout [cout=(h*32+d), cin] -> lhsT [cin, cout2=(4d+h)]"""
    wt = np.asarray(w).reshape(H, D, -1)
    return np.ascontiguousarray(np.transpose(wt, (2, 1, 0)).reshape(-1, H * D))


def _even_up(x):
    x = max(int(x), 2)
    return x + (x & 1)


def prep_inputs(h_X, h_E, mask_attn, W_Q, W_K, W_V, W_O):
    h_X = np.asarray(h_X, dtype=np.float32)
    h_E = np.asarray(h_E, dtype=np.float32)
    mask_attn = np.asarray(mask_attn)
    W_Q = np.asarray(W_Q, dtype=np.float32)
    W_K = np.asarray(W_K, dtype=np.float32)
    W_V = np.asarray(W_V, dtype=np.float32)
    W_O = np.asarray(W_O, dtype=np.float32)

    B, N, Kn, Cin = h_E.shape
    BN = B * N
    nloc = BN // NCORES
    nchunks = nloc // CN

    m = (mask_attn.reshape(BN, Kn) > 0)
    cnt = m.sum(axis=1)
    order = np.argsort(cnt, kind="stable")        # global sorted node ids
    gchunk = NCORES * CN                          # nodes per global chunk

    # adaptive per-chunk capacity (same for every core)
    sched = tuple(_even_up(cnt[order[(ci + 1) * gchunk - 1]])
                  for ci in range(nchunks))

    # neighbor compaction indices: unmasked neighbors first
    jsel = np.argsort(~m, axis=1, kind="stable")  # [BN, K] unmasked first
    msort = np.take_along_axis(m, jsel, axis=1)   # [BN, K] descending 1s

    bf = ml_dtypes.bfloat16
    wq = _perm_dh(W_Q / np.sqrt(D)).astype(bf)
    wk = _perm_dh(W_K).astype(bf)
    wv = _perm_dh(W_V).astype(bf)

    idx = np.arange(C)
    hh = idx % H
    hrep = (hh[:, None] == hh[None, :]).astype(bf)
    ident = np.eye(C, dtype=np.float32).astype(bf)

    wos = W_O[:, :C] + W_O[:, C:2 * C]
    wo3 = W_O[:, 2 * C:]
    wost = np.ascontiguousarray(
        wos.T.reshape(H, D, C).transpose(1, 0, 2).reshape(C, C)).astype(bf)
    wo3t = np.ascontiguousarray(
        wo3.T.reshape(H, D, C).transpose(1, 0, 2).reshape(C, C)).astype(bf)

    hE = h_E.reshape(BN, Kn, Cin)
    xf = h_X.reshape(BN, -1)

    in_maps = []
    for core in range(NCORES):
        nid = order[core::NCORES]                 # this core's nodes, sorted
        et_parts = []
        for ch in range(nchunks):
            kj = sched[ch]
            nd = nid[ch * CN:(ch + 1) * CN]       # [CN]
            sel = jsel[nd][:, :kj]                # [CN, kj]
            g = hE[nd[:, None], sel]              # [CN, kj, Cin]
            g = g * msort[nd][:, :kj, None]       # zero pads
            # [CN, kj, Cin] -> [Cin, kj, CN]
            et_parts.append(g.transpose(2, 1, 0).reshape(Cin, kj * CN))
        etc = np.ascontiguousarray(np.concatenate(et_parts, axis=1)).astype(bf)
        xtc = np.ascontiguousarray(xf[nid].T).astype(bf)
        mcv = np.concatenate(
            [cnt[nid[ch * CN:(ch + 1) * CN]].astype(np.float32) - sched[ch]
             for ch in range(nchunks)])
        mcc = np.ascontiguousarray(
            np.broadcast_to(mcv, (C, nloc)).astype(bf))
        in_maps.append({
            "et": etc, "xt": xtc, "wq": wq, "wk": wk, "wv": wv,
            "hr": hrep, "idn": ident, "wos": wost, "wo3": wo3t, "mc": mcc,
        })
    return in_maps, sched, order


def assemble_output(results, B, N, order):
    BN = B * N
    nloc = BN // NCORES
    outf = np.empty((BN, C), np.float32)
    for core, r in enumerate(results):
        outf[order[core::NCORES]] = r["out"].T
    return outf.reshape(B, N, C)


def kernel(h_X, h_E, mask_attn, W_Q, W_K, W_V, W_O):
    in_maps, sched, order = prep_inputs(
        h_X, h_E, mask_attn, W_Q, W_K, W_V, W_O)
    nc = build_nc(sched)
    res = run_bass_kernel_spmd(nc, in_maps, core_ids=list(range(NCORES)))
    B, N = np.asarray(h_X).shape[:2]
    return assemble_output(res.results, B, N, order)


# revision 65
# speedup vs baseline: 1.0258x; 1.0258x over previous
"""NeighborAttention (B=4, N=4096, K=32, C=128, H=4) on 8 Trainium2 cores.

v4 design (neighbor compaction + engine rebalance), ~141us vs the
272.9us v2 baseline:

  Host packs each node's unmasked neighbors first (attention is
  permutation-invariant over neighbors), sorts nodes globally by
  neighbor count, and deals them round-robin to the 8 cores. Chunks of
  CN=512 nodes get an adaptive per-chunk neighbor capacity K' (even,
  typically (14,16,18,26)) so ~42% of all per-neighbor compute and DMA
  disappears versus uniform K=32. Chunks run largest-K' first.

  Per chunk (j-major layout [c, j, n], bf16, K'=kj):
    AB: PE kt pairs = Wk'@et ([C,1024] PSUM tiles); DVE prod = kt*q
        (1024-wide, q duplicated to [C,1024]); PE s pairs = Hrep@prod
        and ACT e = exp(s) (1024-wide) interleaved one pair behind.
        Score pairs borrow the kt PSUM ring (bufs=3, slots alternate
        kt -> s usage) to stay within the 8 PSUM banks.
    CD: PE vt pairs = Wv'@et; ACT copies vt->SBUF bf16 so the DVE
        uv = e*vt multiply runs in 2x mode (first/last pairs read PSUM
        directly to avoid the ACT queue backlog); PE z += I@e_j and
        usum += I@uv_j identity-accumulations interleaved; the pad
        correction (cnt-K') is folded into the z accumulation as one
        extra identity matmul.
    tail (deferred past the next chunk's AB for overlap):
        DVE in-place bf16 max tree over uv slabs -> umax;
        zc = clamp(z); rz = reciprocal_approx_accurate(zc);
        wsn = usum*rz, mxn = umax*rz (per-head 1/z scaling must precede
        W_O, which mixes heads); PE o = Wos@wsn + Wo3@mxn; ACT copy;
        DMA out.

  attn sums to exactly 1 so the mean/sum W_O blocks fold on the host;
  compaction pad slots have et=0 and contribute exp(0)=1 to z, removed
  by the folded correction term. Each chunk's q is projected one chunk
  ahead (chunk 0's in the prologue) so its ACT copies precede the vc
  backlog. The final chunk's max tree writes level 1 into the dead et
  tile so it does not WAR-wait on the PE usum matmuls still reading uv.
"""
import numpy as np
import ml_dtypes
import concourse.bass as bass
import concourse.bacc as bacc
import concourse.mybir as mybir
from concourse import tile
from concourse.bass_utils import run_bass_kernel_spmd

F32 = mybir.dt.float32
BF16 = mybir.dt.bfloat16
EXP = mybir.ActivationFunctionType.Exp

K = 32
C = 128
H = 4
D = 32
NCORES = 8
CN = 512              # nodes per chunk

_NC_CACHE = {}


def build_nc(sched):
    """sched: tuple of per-chunk neighbor capacities (even ints)."""
    key = tuple(sched)
    if key in _NC_CACHE:
        return _NC_CACHE[key]
    nchunks = len(sched)
    nloc = nchunks * CN
    ncols = sum(sched) * CN
    offs = np.cumsum([0] + [k * CN for k in sched]).tolist()

    nc = bacc.Bacc()
    et_d = nc.dram_tensor("et", [C, ncols], BF16, kind="ExternalInput")
    xt_d = nc.dram_tensor("xt", [C, nloc], BF16, kind="ExternalInput")
    wq_d = nc.dram_tensor("wq", [C, C], BF16, kind="ExternalInput")
    wk_d = nc.dram_tensor("wk", [C, C], BF16, kind="ExternalInput")
    wv_d = nc.dram_tensor("wv", [C, C], BF16, kind="ExternalInput")
    hr_d = nc.dram_tensor("hr", [C, C], BF16, kind="ExternalInput")
    id_d = nc.dram_tensor("idn", [C, C], BF16, kind="ExternalInput")
    wos_d = nc.dram_tensor("wos", [C, C], BF16, kind="ExternalInput")
    wo3_d = nc.dram_tensor("wo3", [C, C], BF16, kind="ExternalInput")
    mc_d = nc.dram_tensor("mc", [C, nloc], BF16, kind="ExternalInput")
    out_d = nc.dram_tensor("out", [C, nloc], F32, kind="ExternalOutput")

    with tile.TileContext(nc) as tc:
        with tc.tile_pool(name="wts", bufs=1) as wpool, \
             tc.tile_pool(name="glob", bufs=1) as gpool, \
             tc.tile_pool(name="etp", bufs=2) as etpool, \
             tc.tile_pool(name="s1p", bufs=2) as s1pool, \
             tc.tile_pool(name="s2p", bufs=2) as s2pool, \
             tc.tile_pool(name="sm", bufs=2) as smpool, \
             tc.tile_pool(name="pkv", bufs=3, space="PSUM") as pkv, \
             tc.tile_pool(name="psz", bufs=1, space="PSUM") as psz, \
             tc.tile_pool(name="psu", bufs=1, space="PSUM") as psu:

            w_q = wpool.tile([C, C], BF16, tag="wq")
            w_k = wpool.tile([C, C], BF16, tag="wk")
            w_v = wpool.tile([C, C], BF16, tag="wv")
            w_h = wpool.tile([C, C], BF16, tag="wh")
            w_i = wpool.tile([C, C], BF16, tag="wi")
            w_os = wpool.tile([C, C], BF16, tag="wos")
            w_o3 = wpool.tile([C, C], BF16, tag="wo3")
            xt_sb = gpool.tile([C, nloc], BF16, tag="xt")
            mc_sb = gpool.tile([C, nloc], BF16, tag="mc")

            # chunk processing order: largest K' first, so the biggest
            # tree/epilogue tails overlap later chunks and the final
            # exposed tail is the smallest one
            corder = sorted(range(nchunks), key=lambda c: -sched[c])

            # DMA priority: what phase A of the first chunk needs comes
            # first
            nc.sync.dma_start(w_q[:], wq_d[:])
            nc.sync.dma_start(xt_sb[:], xt_d[:])
            nc.sync.dma_start(w_k[:], wk_d[:])

            def load_et(ch, pieces=2):
                kj = sched[ch]
                et_sb = etpool.tile([C, kj * CN], BF16, tag="et")
                w = kj * CN
                bnds = [w * i // pieces for i in range(pieces + 1)]
                for a, b in zip(bnds, bnds[1:]):
                    nc.sync.dma_start(et_sb[:, a:b],
                                      et_d[:, offs[ch] + a:offs[ch] + b])
                return et_sb

            et_tiles = {corder[0]: load_et(corder[0], pieces=4)}
            for t, dd in ((w_h, hr_d), (w_v, wv_d), (w_i, id_d),
                          (w_os, wos_d), (w_o3, wo3_d)):
                nc.sync.dma_start(t[:], dd[:])
            nc.sync.dma_start(mc_sb[:], mc_d[:])

            # q projection for a chunk, duplicated to [C, 2CN] so the
            # paired 1024-wide prod multiplies can broadcast it over both
            # slabs; issued one chunk ahead so the ACT copies precede the
            # vc backlog (chunk 0's is issued here in the prologue)
            def emit_q(c2):
                q_ps = psz.tile([C, CN], F32, tag="z", name="q_ps")
                nc.tensor.matmul(q_ps[:], w_q[:],
                                 xt_sb[:, c2 * CN:(c2 + 1) * CN],
                                 start=True, stop=True)
                q2 = smpool.tile([C, 2 * CN], F32, tag="q", bufs=2,
                                 name="q2")
                nc.scalar.copy(q2[:, :CN], q_ps[:])
                nc.scalar.copy(q2[:, CN:], q_ps[:])
                return q2

            q_next = emit_q(corder[0])
            pending_tail = None

            for ci in range(nchunks):
                ch = corder[ci]
                kj = sched[ch]
                n0 = ch * CN
                et_sb = et_tiles.pop(ch)
                if ci + 1 < nchunks:
                    et_tiles[corder[ci + 1]] = load_et(corder[ci + 1])

                q_sb = q_next

                prod = s1pool.tile([C, kj * CN], BF16, tag="s1")
                e_ch = s2pool.tile([C, kj * CN], BF16, tag="s2")

                # AB: k-projection (paired into [C,1024] PSUM tiles, one
                # wide DVE multiply per pair) with score matmuls + exp
                # interleaved one pair behind. Score pairs borrow the same
                # kv PSUM ring (each slot alternates kt -> s usage), which
                # keeps exp 1024-wide within the 8-bank budget.
                def emit_score_pair(j):
                    s_ps = pkv.tile([C, 2 * CN], F32, tag="kv")
                    nc.tensor.matmul(s_ps[:, :CN], w_h[:],
                                     prod[:, j * CN:(j + 1) * CN],
                                     start=True, stop=True)
                    nc.tensor.matmul(s_ps[:, CN:], w_h[:],
                                     prod[:, (j + 1) * CN:(j + 2) * CN],
                                     start=True, stop=True)
                    nc.scalar.activation(e_ch[:, j * CN:(j + 2) * CN],
                                         s_ps[:], EXP)

                for jp in range(kj // 2):
                    j = 2 * jp
                    kt2 = pkv.tile([C, 2 * CN], F32, tag="kv")
                    nc.tensor.matmul(kt2[:, :CN], w_k[:],
                                     et_sb[:, j * CN:(j + 1) * CN],
                                     start=True, stop=True)
                    nc.tensor.matmul(kt2[:, CN:], w_k[:],
                                     et_sb[:, (j + 1) * CN:(j + 2) * CN],
                                     start=True, stop=True)
                    nc.vector.tensor_mul(prod[:, j * CN:(j + 2) * CN],
                                         kt2[:], q_sb[:])
                    if jp >= 1:
                        emit_score_pair(j - 2)
                emit_score_pair(kj - 2)

                # previous chunk's max tree + epilogue runs here, overlapped
                # with this chunk's CD phase
                if pending_tail is not None:
                    pending_tail()
                if ci + 1 < nchunks:
                    q_next = emit_q(corder[ci + 1])

                # CD: v-projection (paired), ACT PSUM->SBUF bf16 copy so the
                # DVE multiply runs in 2x mode, z/usum identity accumulation
                # interleaved
                uv = prod
                z_ps = psz.tile([C, CN], F32, tag="z")
                u_ps = psu.tile([C, CN], F32, tag="u")

                npair = kj // 2
                for jp in range(npair):
                    j = 2 * jp
                    vt2 = pkv.tile([C, 2 * CN], F32, tag="kv")
                    nc.tensor.matmul(vt2[:, :CN], w_v[:],
                                     et_sb[:, j * CN:(j + 1) * CN],
                                     start=True, stop=True)
                    nc.tensor.matmul(vt2[:, CN:], w_v[:],
                                     et_sb[:, (j + 1) * CN:(j + 2) * CN],
                                     start=True, stop=True)
                    if jp == 0:
                        # direct PSUM multiply: no ACT-queue dependency, so
                        # the DVE starts immediately and the kv ring frees
                        # without waiting on the (backlogged) ACT copies
                        nc.vector.tensor_mul(uv[:, j * CN:(j + 2) * CN],
                                             e_ch[:, j * CN:(j + 2) * CN],
                                             vt2[:])
                    else:
                        vc = smpool.tile([C, 2 * CN], BF16, tag="vc", bufs=4)
                        nc.scalar.copy(vc[:], vt2[:])
                        nc.vector.tensor_mul(uv[:, j * CN:(j + 2) * CN],
                                             e_ch[:, j * CN:(j + 2) * CN],
                                             vc[:])
                    nc.tensor.matmul(z_ps[:], w_i[:],
                                     e_ch[:, j * CN:(j + 1) * CN],
                                     start=(j == 0), stop=False,
                                     skip_group_check=True)
                    nc.tensor.matmul(z_ps[:], w_i[:],
                                     e_ch[:, (j + 1) * CN:(j + 2) * CN],
                                     start=False, stop=False,
                                     skip_group_check=True)
                    if jp >= 1:
                        nc.tensor.matmul(u_ps[:], w_i[:],
                                         uv[:, (j - 2) * CN:(j - 1) * CN],
                                         start=(jp == 1), stop=False,
                                         skip_group_check=True)
                        nc.tensor.matmul(u_ps[:], w_i[:],
                                         uv[:, (j - 1) * CN:j * CN],
                                         start=False, stop=False,
                                         skip_group_check=True)
                nc.tensor.matmul(u_ps[:], w_i[:],
                                 uv[:, (kj - 2) * CN:(kj - 1) * CN],
                                 start=(kj == 2), stop=False,
                                 skip_group_check=True)
                nc.tensor.matmul(u_ps[:], w_i[:],
                                 uv[:, (kj - 1) * CN:kj * CN],
                                 start=False, stop=True,
                                 skip_group_check=True)

                # fold the pad correction into the z accumulation:
                # mc holds (cnt - K') so this subtracts the pad count
                nc.tensor.matmul(z_ps[:], w_i[:], mc_sb[:, n0:n0 + CN],
                                 start=False, stop=True,
                                 skip_group_check=True)

                def make_tail(kj, n0, uv, z_ps, u_ps, scratch=None):
                    def tail():
                        # T: max tree over uv slabs (DVE, bf16 2x). For the
                        # final chunk the first level writes into the dead et
                        # tile instead of in-place: writing uv would WAR-wait
                        # on the PE usum matmuls still reading it, exposing
                        # ~4us of serial tail at the very end.
                        tre = uv
                        scr = scratch
                        w = kj
                        while w > 1:
                            hw = w // 2
                            dst = scr if scr is not None else tre
                            nc.vector.tensor_max(dst[:, :hw * CN],
                                                 tre[:, :hw * CN],
                                                 tre[:, hw * CN:2 * hw * CN])
                            if w % 2:
                                nc.vector.tensor_max(dst[:, :CN],
                                                     dst[:, :CN],
                                                     tre[:, (w - 1) * CN:w * CN])
                            tre = dst
                            scr = None  # subsequent levels in-place
                            w = hw

                        # E: scale aggregates by 1/z per head BEFORE W_O
                        zcl = smpool.tile([C, CN], F32, tag="zcl", bufs=1)
                        nc.vector.tensor_scalar_max(zcl[:], z_ps[:], 1e-20)
                        rz = smpool.tile([C, CN], F32, tag="rz")
                        nc.vector.reciprocal_approx_fast(out=rz[:],
                                                         in_=zcl[:])

                        wsn = smpool.tile([C, CN], BF16, tag="wsn")
                        nc.vector.tensor_mul(wsn[:], u_ps[:], rz[:])
                        mxn = smpool.tile([C, CN], BF16, tag="mxn")
                        nc.vector.tensor_mul(mxn[:], tre[:, :CN], rz[:])

                        o_ps = psu.tile([C, CN], F32, tag="u")
                        nc.tensor.matmul(o_ps[:], w_os[:], wsn[:],
                                         start=True, stop=False)
                        nc.tensor.matmul(o_ps[:], w_o3[:], mxn[:],
                                         start=False, stop=True)
                        o_sb = smpool.tile([C, CN], F32, tag="osb")
                        nc.scalar.copy(o_sb[:], o_ps[:])
                        nc.sync.dma_start(out_d[:, n0:n0 + CN], o_sb[:])
                    return tail

                pending_tail = make_tail(
                    kj, n0, uv, z_ps, u_ps,
                    scratch=et_sb if ci == nchunks - 1 else None)

            pending_tail()

    nc.compile()
    _NC_CACHE[key] = nc
    return nc


def _perm_dh(w):
    """torch-layout [cout=(h*32+d), cin] -> lhsT [cin, cout2=(4d+h)]"""
    wt = np.asarray(w).reshape(H, D, -1)
    return np.ascontiguousarray(np.transpose(wt, (2, 1, 0)).reshape(-1, H * D))


def _even_up(x):
    x = max(int(x), 2)
    return x + (x & 1)


def prep_inputs(h_X, h_E, mask_attn, W_Q, W_K, W_V, W_O):
    h_X = np.asarray(h_X, dtype=np.float32)
    h_E = np.asarray(h_E, dtype=np.float32)
    mask_attn = np.asarray(mask_attn)
    W_Q = np.asarray(W_Q, dtype=np.float32)
    W_K = np.asarray(W_K, dtype=np.float32)
    W_V = np.asarray(W_V, dtype=np.float32)
    W_O = np.asarray(W_O, dtype=np.float32)

    B, N, Kn, Cin = h_E.shape
    BN = B * N
    nloc = BN // NCORES
    nchunks = nloc // CN

    m = (mask_attn.reshape(BN, Kn) > 0)
    cnt = m.sum(axis=1)
    order = np.argsort(cnt, kind="stable")        # global sorted node ids
    gchunk = NCORES * CN                          # nodes per global chunk

    # adaptive per-chunk capacity (same for every core)
    sched = tuple(_even_up(cnt[order[(ci + 1) * gchunk - 1]])
                  for ci in range(nchunks))

    # neighbor compaction indices: unmasked neighbors first
    jsel = np.argsort(~m, axis=1, kind="stable")  # [BN, K] unmasked first
    msort = np.take_along_axis(m, jsel, axis=1)   # [BN, K] descending 1s

    bf = ml_dtypes.bfloat16
    wq = _perm_dh(W_Q / np.sqrt(D)).astype(bf)
    wk = _perm_dh(W_K).astype(bf)
    wv = _perm_dh(W_V).astype(bf)

    idx = np.arange(C)
    hh = idx % H
    hrep = (hh[:, None] == hh[None, :]).astype(bf)
    ident = np.eye(C, dtype=np.float32).astype(bf)

    wos = W_O[:, :C] + W_O[:, C:2 * C]
    wo3 = W_O[:, 2 * C:]
    wost = np.ascontiguousarray(
        wos.T.reshape(H, D, C).transpose(1, 0, 2).reshape(C, C)).astype(bf)
    wo3t = np.ascontiguousarray(
        wo3.T.reshape(H, D, C).transpose(1, 0, 2).reshape(C, C)).astype(bf)

    hE = h_E.reshape(BN, Kn, Cin)
    xf = h_X.reshape(BN, -1)

    in_maps = []
    for core in range(NCORES):
        nid = order[core::NCORES]                 # this core's nodes, sorted
        et_parts = []
        for ch in range(nchunks):
            kj = sched[ch]
            nd = nid[ch * CN:(ch + 1) * CN]       # [CN]
            sel = jsel[nd][:, :kj]                # [CN, kj]
            g = hE[nd[:, None], sel]              # [CN, kj, Cin]
            g = g * msort[nd][:, :kj, None]       # zero pads
            # [CN, kj, Cin] -> [Cin, kj, CN]
            et_parts.append(g.transpose(2, 1, 0).reshape(Cin, kj * CN))
        etc = np.ascontiguousarray(np.concatenate(et_parts, axis=1)).astype(bf)
        xtc = np.ascontiguousarray(xf[nid].T).astype(bf)
        mcv = np.concatenate(
            [cnt[nid[ch * CN:(ch + 1) * CN]].astype(np.float32) - sched[ch]
             for ch in range(nchunks)])
        mcc = np.ascontiguousarray(
            np.broadcast_to(mcv, (C, nloc)).astype(bf))
        in_maps.append({
            "et": etc, "xt": xtc, "wq": wq, "wk": wk, "wv": wv,
            "hr": hrep, "idn": ident, "wos": wost, "wo3": wo3t, "mc": mcc,
        })
    return in_maps, sched, order


def assemble_output(results, B, N, order):
    BN = B * N
    nloc = BN // NCORES
    outf = np.empty((BN, C), np.float32)
    for core, r in enumerate(results):
        outf[order[core::NCORES]] = r["out"].T
    return outf.reshape(B, N, C)


def kernel(h_X, h_E, mask_attn, W_Q, W_K, W_V, W_O):
    in_maps, sched, order = prep_inputs(
        h_X, h_E, mask_attn, W_Q, W_K, W_V, W_O)
    nc = build_nc(sched)
    res = run_bass_kernel_spmd(nc, in_maps, core_ids=list(range(NCORES)))
    B, N = np.asarray(h_X).shape[:2]
    return assemble_output(res.results, B, N, order)
